# revision 1
# baseline (speedup 1.0000x reference)
"""Fused Linear + LayerNorm + residual-multiply kernel for 8 Trainium2 cores.

Computes, for full inputs x[B,1024], y[B,1024], weight[1024,1024], bias, gamma, beta:
    z  = x @ weight.T + bias
    ln = (z - mean(z)) * rsqrt(var(z) + eps) * gamma + beta     (over last dim)
    out = (ln + y) * y

Data-parallel over the batch dim: each of the 8 NeuronCores processes B/8 rows;
weight/bias/gamma/beta are replicated. No cross-core communication.

Host-side layout/precision prep (like pre-transposing weights): x and W.T are
cast to fp16, x is stored transposed ([in_features, rows]) so the contraction
dim lands on SBUF partitions, and both are packed partition-major so every DMA
descriptor is a multi-KB contiguous run. The matmul accumulates in fp32 PSUM;
everything after the matmul (stats, normalize, residual) is fp32.

Per-core algorithm (b_core = B/8 rows, P=128, D=1024):
  - W.T fp16 resident in SBUF; x.T fp16 streamed in 512-row super-chunks.
  - Per 128-row tile: 8 accumulating fp16 matmuls per 512-wide output block
    (stationary = x.T block, moving = W.T block) plus a K=1 ones x bias matmul
    that adds the bias row inside PSUM.
  - LayerNorm stats on ScalarE via activation accum_out: Copy(z) gives sum(z),
    Square(z) gives sum(z^2); sqrt(sum(z^2)/D - mean^2 + eps) in one Sqrt
    activation; reciprocal on VectorE.  ScalarE Identity-activation applies
    (z - mean) * rstd while copying PSUM -> SBUF; VectorE computes (+y)*y.
  - DMA split over both HWDGE rings: x.T + y on the sync-engine queue,
    W.T + out on the scalar-engine queue.
"""

import numpy as np
from contextlib import ExitStack

import concourse.bass as bass
import concourse.mybir as mybir
import concourse.tile as tile
from concourse import bacc, bass_utils


P = 128
D = 1024
KT = D // P          # 8 k-tiles over the contraction dim
OB = 512             # o-block width (one PSUM bank of fp32)
ST = 512             # rows per x.T super-chunk
N_CORES = 8
EPS = 1e-5

F32 = mybir.dt.float32
F16 = mybir.dt.float16

AF = mybir.ActivationFunctionType
OP = mybir.AluOpType

_BUILD_CACHE = {}


def _build(b_core: int, trivial_affine: bool):
    key = (b_core, trivial_affine)
    if key in _BUILD_CACHE:
        return _BUILD_CACHE[key]

    nst = b_core // ST
    nc = bacc.Bacc("TRN2", debug=False, num_devices=N_CORES)

    # x.T packed as [super-chunk, partition, k * b_local] (contiguous per partition)
    xt = nc.dram_tensor("xt", [nst, P, KT * ST], F16, kind="ExternalInput").ap()
    yh = nc.dram_tensor("yh", [b_core, D], F16, kind="ExternalInput").ap()
    # W.T packed as [k, partition, o] (contiguous per partition per k-block)
    wth = nc.dram_tensor("wth", [KT, P, D], F16, kind="ExternalInput").ap()
    biash = nc.dram_tensor("biash", [D], F16, kind="ExternalInput").ap()
    if not trivial_affine:
        gamma = nc.dram_tensor("gamma", [D], F32, kind="ExternalInput").ap()
        beta = nc.dram_tensor("beta", [D], F32, kind="ExternalInput").ap()
    out = nc.dram_tensor("out", [b_core, D], F32, kind="ExternalOutput").ap()

    with tile.TileContext(nc) as tc, ExitStack() as ctx:
        const = ctx.enter_context(tc.tile_pool(name="const", bufs=1))
        xtp = ctx.enter_context(tc.tile_pool(name="xtp", bufs=2))
        ypool = ctx.enter_context(tc.tile_pool(name="yp", bufs=4))
        tpool = ctx.enter_context(tc.tile_pool(name="tp", bufs=3))
        spool = ctx.enter_context(tc.tile_pool(name="sq", bufs=2))
        opool = ctx.enter_context(tc.tile_pool(name="op", bufs=3))
        stat = ctx.enter_context(tc.tile_pool(name="stat", bufs=6))
        psz = ctx.enter_context(tc.tile_pool(name="psz", bufs=4, space="PSUM"))

        # --- constants ---
        bias_sb = const.tile([1, D], F16)
        nc.scalar.dma_start(out=bias_sb[:], in_=biash.unsqueeze(0))
        wt_sb = const.tile([P, KT, D], F16)  # [i_local, k, o]
        for k in range(KT):
            nc.scalar.dma_start(out=wt_sb[:, k, :], in_=wth[k])
        ones_f32 = const.tile([1, P], F32)
        nc.vector.memset(ones_f32[:], 1.0)
        ones_sb = const.tile([1, P], F16)
        nc.scalar.activation(ones_sb[:], ones_f32[:], AF.Copy)
        eps_sb = const.tile([P, 1], F32)
        nc.vector.memset(eps_sb[:], EPS)
        if not trivial_affine:
            gamma_f32 = const.tile([P, D], F32)
            nc.sync.dma_start(out=gamma_f32[:], in_=gamma.unsqueeze(0).to_broadcast([P, D]))
            gamma_sb = const.tile([P, D], F16)
            nc.scalar.activation(gamma_sb[:], gamma_f32[:], AF.Copy)
            beta_f32 = const.tile([P, D], F32)
            nc.sync.dma_start(out=beta_f32[:], in_=beta.unsqueeze(0).to_broadcast([P, D]))
            beta_sb = const.tile([P, D], F16)
            nc.scalar.activation(beta_sb[:], beta_f32[:], AF.Copy)

        # --- PE warmup: keep the HAM activity monitor busy during input
        # staging so the real matmuls start at 2.4 GHz instead of 1.2 GHz.
        # fp32 matmuls stream 4 cyc/row, so 3 of them cover the ~3.4us window.
        warm_mov = const.tile([1, OB], F32)
        nc.vector.memset(warm_mov[:], 0.0)
        warm_ps = psz.tile([P, D], F32, tag="z_ps")
        for w in range(2):
            nc.tensor.matmul(
                warm_ps[:, 0:OB], ones_f32[:], warm_mov[:], start=True, stop=True
            )

        nb = b_core // P
        for bt in range(nb):
            if bt % (ST // P) == 0:
                st = bt // (ST // P)
                xt_sb = xtp.tile([P, KT, ST], F16)  # [i_local, k, b_local]
                xt_src = xt[st].rearrange("p (k b) -> p k b", k=KT)
                # k-split loads: matmul k only waits for x.T block k
                for k in range(KT):
                    nc.sync.dma_start(
                        out=xt_sb[:, k, :], in_=xt_src[:, k, :]
                    )
            j = bt % (ST // P)
            rows = slice(bt * P, (bt + 1) * P)
            y_sb = ypool.tile([P, D], F16)
            nc.sync.dma_start(out=y_sb[:], in_=yh[rows, :])

            # --- matmuls: z = x @ W.T + bias, accumulated in PSUM ---
            # k-outer order so each matmul only needs W.T/x.T block k loaded
            # (ISA caps one matmul at 512 output elements = one PSUM bank).
            z_ps = psz.tile([P, D], F32)
            stt = stat.tile([P, 2, 6], F32)
            for k in range(KT):
                lhsT = xt_sb[:, k, bass.ts(j, P)]
                for half in range(2):
                    nc.tensor.matmul(
                        z_ps[:, bass.ts(half, OB)],
                        lhsT,
                        wt_sb[:, k, bass.ts(half, OB)],
                        start=(k == 0),
                        stop=False,
                    )
            for half in range(2):
                nc.tensor.matmul(
                    z_ps[:, bass.ts(half, OB)],
                    ones_sb[:],
                    bias_sb[:, bass.ts(half, OB)],
                    start=False,
                    stop=True,
                )
                nc.vector.bn_stats(
                    out=stt[:, half, :], in_=z_ps[:, bass.ts(half, OB)]
                )

            mv = stat.tile([P, 2], F32)
            nc.vector.bn_aggr(out=mv[:], in_=stt[:])
            std = stat.tile([P, 1], F32)
            nc.scalar.activation(std[:], mv[:, 1:2], AF.Sqrt, bias=eps_sb[:], scale=1.0)
            rstd = stat.tile([P, 1], F32)
            nc.vector.reciprocal(rstd[:], std[:])
            nmr = stat.tile([P, 1], F32)  # -mean * rstd
            nc.vector.scalar_tensor_tensor(
                out=nmr[:], in0=mv[:, 0:1], scalar=-1.0, in1=rstd[:],
                op0=OP.mult, op1=OP.mult,
            )

            # --- normalize + residual: t = (z-mean)*rstd; out = (t+y)*y ---
            # (fp16 tensor ops run in DVE 2x mode; ScalarE casts back to fp32)
            # The last tile runs the chain in halves to shorten the drain tail.
            t_sb = tpool.tile([P, D], F16)
            u_sb = spool.tile([P, D], F16)
            o_sb = opool.tile([P, D], F16)
            o32_sb = opool.tile([P, D], F32)
            chunks = 2 if bt == nb - 1 else 1
            cw = D // chunks
            for q in range(chunks):
                cs = bass.ts(q, cw)
                nc.scalar.activation(
                    t_sb[:, cs], z_ps[:, cs], AF.Identity, bias=nmr[:], scale=rstd[:]
                )
                if not trivial_affine:
                    nc.vector.tensor_mul(out=t_sb[:, cs], in0=t_sb[:, cs], in1=gamma_sb[:, cs])
                    nc.vector.tensor_add(out=t_sb[:, cs], in0=t_sb[:, cs], in1=beta_sb[:, cs])
                nc.vector.tensor_add(out=u_sb[:, cs], in0=t_sb[:, cs], in1=y_sb[:, cs])
                nc.vector.tensor_mul(out=o_sb[:, cs], in0=u_sb[:, cs], in1=y_sb[:, cs])
                nc.scalar.activation(o32_sb[:, cs], o_sb[:, cs], AF.Copy)
                nc.scalar.dma_start(out=out[rows, cs], in_=o32_sb[:, cs])

    nc.finalize()
    _BUILD_CACHE[key] = nc
    return nc


def _run(nc, in_maps, **kwargs):
    return bass_utils.run_bass_kernel_spmd(
        nc, in_maps, core_ids=list(range(N_CORES)), **kwargs
    )


def _prepare(x, y, weight, bias, gamma, beta):
    x = np.asarray(x, dtype=np.float32)
    y = np.ascontiguousarray(y, dtype=np.float32)
    weight = np.asarray(weight, dtype=np.float32)
    bias = np.asarray(bias, dtype=np.float32)
    gamma = np.asarray(gamma, dtype=np.float32)
    beta = np.asarray(beta, dtype=np.float32)

    B, IN = x.shape
    assert IN == D and weight.shape == (D, D) and y.shape == (B, D)
    assert B % (N_CORES * ST) == 0
    b_core = B // N_CORES
    nst = b_core // ST

    trivial = bool(np.all(gamma == 1.0)) and bool(np.all(beta == 0.0))
    nc = _build(b_core, trivial)

    # W.T packed: wth_prep[k, p, o] = W.T[k*P + p, o]
    wth_prep = np.ascontiguousarray(
        weight.T.astype(np.float16).reshape(KT, P, D)
    )
    biash = bias.astype(np.float16)
    in_maps = []
    for c in range(N_CORES):
        xs = x[c * b_core:(c + 1) * b_core].astype(np.float16)
        # x.T packed: xt_prep[st, p, k, b_local] = x.T[k*P + p, st*ST + b_local]
        xt_prep = np.ascontiguousarray(
            xs.T.reshape(KT, P, nst, ST).transpose(2, 1, 0, 3)
        ).reshape(nst, P, KT * ST)
        m = {
            "xt": xt_prep,
            "yh": np.ascontiguousarray(
                y[c * b_core:(c + 1) * b_core].astype(np.float16)
            ),
            "wth": wth_prep,
            "biash": biash,
        }
        if not trivial:
            m["gamma"] = gamma
            m["beta"] = beta
        in_maps.append(m)
    return nc, in_maps


def kernel(x, y, weight, bias, gamma, beta):
    nc, in_maps = _prepare(x, y, weight, bias, gamma, beta)
    res = _run(nc, in_maps)
    return np.concatenate([r["out"] for r in res.results], axis=0)



# revision 2
# speedup vs baseline: 1.1469x; 1.1469x over previous
"""Fused Linear + LayerNorm + residual-multiply kernel for 8 Trainium2 cores.

Computes, for full inputs x[B,1024], y[B,1024], weight[1024,1024], bias, gamma, beta:
    z  = x @ weight.T + bias
    ln = (z - mean(z)) * rsqrt(var(z) + eps) * gamma + beta     (over last dim)
    out = (ln + y) * y

Data-parallel over the batch dim: each of the 8 NeuronCores processes B/8 rows;
weight/bias/gamma/beta are replicated. No cross-core communication.

Fast path (gamma==1, beta==0), built around three ideas:

1. fp8 DoubleRow matmuls. x and W.T are quantized to fp8-e4m3 on the host
   (W.T pre-scaled by 32 so its entries use the e4m3 normal range; LayerNorm
   is scale-invariant so only eps needs compensating: eps' = 32^2 * eps).
   DoubleRow packs two contraction rows per PE cell -> K=256 per matmul,
   halving the matmul instruction count vs fp16. PSUM accumulates fp32.
   The per-row bias is added by a K=1 ones x (32*bias) fp16 matmul pair
   issued with start=True before the x arrives.

2. One PSUM read + fused fp16 consumer. ScalarE copies z' = 32*z out of PSUM
   to fp16 (accum_out gives sum(z') for free). VectorE computes sum(z'^2) with
   a scalar_tensor_tensor on the fp16 copy (2x DVE mode), the variance smalls
   run on ScalarE ([P,1] activations), and the whole normalize+residual
   collapses into two DVE STT ops:
       w   = (t0 * rstd') + y          (rstd' = rstd/32, per-partition scalar)
       out = (w + c) * y               (c = -mean' * rstd', per-partition)
   Output is written fp16 and upcast on the host.

3. Batched DMA. Every tensor moves as one descriptor set per 512-row chunk
   (contiguous multi-KB per partition, packed on the host), cutting HWDGE
   issue cost ~4x. Inputs ride the sync-engine ring, outputs the
   scalar-engine ring.
"""

import numpy as np
import ml_dtypes
from contextlib import ExitStack

import concourse.bass as bass
import concourse.mybir as mybir
import concourse.tile as tile
from concourse import bacc, bass_utils


P = 128
D = 1024
KT = D // P          # 8 k-tiles over the contraction dim
KK = KT // 2         # 4 DoubleRow pairs
OB = 512             # o-block width (one PSUM bank of fp32)
ST = 512             # rows per super-chunk
TPC = ST // P        # 4 tiles per chunk
N_CORES = 8
EPS = 1e-5
W_SCALE = 32.0       # W.T pre-scale so fp8 entries stay in normal range
EPS_DEV = EPS * W_SCALE * W_SCALE   # eps seen by the scaled z' = 32z
SQ_SCALE = 0.25      # sq pass computes (t0*0.25)*t0 -> sumsq' = 4 * accum

F32 = mybir.dt.float32
F16 = mybir.dt.float16
F8 = mybir.dt.float8e4

AF = mybir.ActivationFunctionType
OP = mybir.AluOpType
DR = mybir.MatmulPerfMode.DoubleRow

_BUILD_CACHE = {}


def _build(b_core: int, trivial_affine: bool):
    key = (b_core, trivial_affine)
    if key in _BUILD_CACHE:
        return _BUILD_CACHE[key]

    nst = b_core // ST
    nc = bacc.Bacc("TRN2", debug=False, num_devices=N_CORES)

    # x.T packed fp8: xt[st, p, k*ST + b] = x.T[k*P + p, st*ST + b]
    xt = nc.dram_tensor("xt", [nst, P, KT * ST], F8, kind="ExternalInput").ap()
    # y packed fp16: yh[st, p, t*D + o] = y[st*ST + t*P + p, o]
    yh = nc.dram_tensor("yh", [nst, P, TPC * D], F16, kind="ExternalInput").ap()
    # W.T packed fp8 (pre-scaled by 32): wth[p, k*D + o] = 32 * W.T[k*P + p, o]
    wth = nc.dram_tensor("wth", [P, KT * D], F8, kind="ExternalInput").ap()
    biash = nc.dram_tensor("biash", [D], F16, kind="ExternalInput").ap()  # 32*bias
    if not trivial_affine:
        gamma = nc.dram_tensor("gamma", [D], F32, kind="ExternalInput").ap()
        beta = nc.dram_tensor("beta", [D], F32, kind="ExternalInput").ap()
    # out packed fp16: outh[st, p, t*D + o] = out[st*ST + t*P + p, o]
    outh = nc.dram_tensor("outh", [nst, P, TPC * D], F16, kind="ExternalOutput").ap()

    with tile.TileContext(nc) as tc, ExitStack() as ctx:
        const = ctx.enter_context(tc.tile_pool(name="const", bufs=1))
        xtp = ctx.enter_context(tc.tile_pool(name="xtp", bufs=2))
        ypool = ctx.enter_context(tc.tile_pool(name="yp", bufs=2))
        tpool = ctx.enter_context(tc.tile_pool(name="tp", bufs=3))
        jpool = ctx.enter_context(tc.tile_pool(name="jp", bufs=2))
        wpool = ctx.enter_context(tc.tile_pool(name="wp", bufs=3))
        opool = ctx.enter_context(tc.tile_pool(name="op", bufs=2))
        stat = ctx.enter_context(tc.tile_pool(name="stat", bufs=8))
        psz = ctx.enter_context(tc.tile_pool(name="psz", bufs=4, space="PSUM"))

        # --- constants ---
        bias_sb = const.tile([1, D], F16)
        nc.scalar.dma_start(out=bias_sb[:], in_=biash.unsqueeze(0))
        wt_sb = const.tile([P, KT, D], F8)  # [i_local, k, o]
        wt_src = wth.rearrange("p (k o) -> p k o", k=KT)
        # halves so kk 0/1 matmuls only wait on the first half
        nc.sync.dma_start(out=wt_sb[:, 0 : KT // 2, :], in_=wt_src[:, 0 : KT // 2, :])
        nc.sync.dma_start(out=wt_sb[:, KT // 2 :, :], in_=wt_src[:, KT // 2 :, :])
        ones_f32 = const.tile([1, P], F32)
        nc.vector.memset(ones_f32[:], 1.0)
        ones_sb = const.tile([1, P], F16)
        nc.scalar.activation(ones_sb[:], ones_f32[:], AF.Copy)
        eps_sb = const.tile([P, 1], F32)
        nc.vector.memset(eps_sb[:], EPS_DEV)
        if not trivial_affine:
            gamma_f32 = const.tile([P, D], F32)
            nc.sync.dma_start(out=gamma_f32[:], in_=gamma.unsqueeze(0).to_broadcast([P, D]))
            gamma_sb = const.tile([P, D], F16)
            nc.scalar.activation(gamma_sb[:], gamma_f32[:], AF.Copy)
            beta_f32 = const.tile([P, D], F32)
            nc.sync.dma_start(out=beta_f32[:], in_=beta.unsqueeze(0).to_broadcast([P, D]))
            beta_sb = const.tile([P, D], F16)
            nc.scalar.activation(beta_sb[:], beta_f32[:], AF.Copy)

        # --- PE warmup: keep the HAM activity monitor busy during input
        # staging so the real matmuls start at 2.4 GHz instead of 1.2 GHz.
        warm_mov = const.tile([1, OB], F32)
        nc.vector.memset(warm_mov[:], 0.0)
        warm_ps = psz.tile([P, D], F32, tag="z_ps")
        for w in range(2):
            nc.tensor.matmul(
                warm_ps[:, 0:OB], ones_f32[:], warm_mov[:], start=True, stop=True
            )

        for st in range(nst):
            xt_sb = xtp.tile([P, KT, ST], F8)  # [i_local, k, b_local]
            nc.sync.dma_start(
                out=xt_sb[:], in_=xt[st].rearrange("p (k b) -> p k b", k=KT)
            )
            y_sb = ypool.tile([P, TPC, D], F16)
            nc.sync.dma_start(
                out=y_sb[:], in_=yh[st].rearrange("p (t o) -> p t o", t=TPC)
            )
            o_sb = opool.tile([P, TPC, D], F16)

            for t in range(TPC):
                rows_t = bass.ts(t, P)
                z_ps = psz.tile([P, D], F32)

                # --- matmuls: z' = 32*(x @ W.T + bias) in PSUM fp32 ---
                # bias first (start=True) so it can issue before x arrives
                for half in range(2):
                    nc.tensor.matmul(
                        z_ps[:, bass.ts(half, OB)],
                        ones_sb[:],
                        bias_sb[:, bass.ts(half, OB)],
                        start=True,
                        stop=False,
                    )
                for kk in range(KK):
                    ksl = slice(2 * kk, 2 * kk + 2)
                    lhsT = xt_sb[:, ksl, rows_t]
                    for half in range(2):
                        nc.tensor.matmul(
                            z_ps[:, bass.ts(half, OB)],
                            lhsT,
                            wt_sb[:, ksl, bass.ts(half, OB)],
                            start=False,
                            stop=(kk == KK - 1),
                            perf_mode=DR,
                        )

                y_t = y_sb[:, t, :]
                if trivial_affine:
                    # --- single PSUM read: t0 = fp16(z'), sm = sum(z') ---
                    t0 = tpool.tile([P, D], F16)
                    sm = stat.tile([P, 1], F32)
                    nc.scalar.activation(t0[:], z_ps[:], AF.Copy, accum_out=sm[:])
                    # ssq = sum(t0^2)/4 on DVE (fp16 2x mode)
                    junk = jpool.tile([P, D], F16)
                    ssq = stat.tile([P, 1], F32)
                    nc.vector.scalar_tensor_tensor(
                        out=junk[:], in0=t0[:], scalar=SQ_SCALE, in1=t0[:],
                        op0=OP.mult, op1=OP.mult, accum_out=ssq[:],
                    )
                    # variance smalls on ScalarE:
                    #   q   = sm^2 / D
                    #   v2  = q - 4*ssq            = -(var' * D)
                    #   std = sqrt(-v2/D + eps')   = sqrt(var' + eps')
                    q = stat.tile([P, 1], F32)
                    nc.scalar.activation(
                        q[:], sm[:], AF.Square, scale=float(1.0 / np.sqrt(D))
                    )
                    v2 = stat.tile([P, 1], F32)
                    nc.scalar.activation(
                        v2[:], ssq[:], AF.Identity, scale=-1.0 / SQ_SCALE, bias=q[:]
                    )
                    std = stat.tile([P, 1], F32)
                    nc.scalar.activation(
                        std[:], v2[:], AF.Sqrt, scale=-1.0 / D, bias=eps_sb[:]
                    )
                    rstd = stat.tile([P, 1], F32)
                    nc.vector.reciprocal(rstd[:], std[:])
                    c = stat.tile([P, 1], F32)
                    nc.vector.scalar_tensor_tensor(
                        out=c[:], in0=sm[:], scalar=-1.0 / D, in1=rstd[:],
                        op0=OP.mult, op1=OP.mult,
                    )
                    # --- fused normalize + residual: out = ((t0*rstd + y) + c) * y
                    w_sb = wpool.tile([P, D], F16)
                    nc.vector.scalar_tensor_tensor(
                        out=w_sb[:], in0=t0[:], scalar=rstd[:], in1=y_t,
                        op0=OP.mult, op1=OP.add,
                    )
                    nc.vector.scalar_tensor_tensor(
                        out=o_sb[:, t, :], in0=w_sb[:], scalar=c[:], in1=y_t,
                        op0=OP.add, op1=OP.mult,
                    )
                else:
                    # general path: bn_stats on PSUM, scalar normalize, affine
                    stt = stat.tile([P, 2, 6], F32)
                    for half in range(2):
                        nc.vector.bn_stats(
                            out=stt[:, half, :], in_=z_ps[:, bass.ts(half, OB)]
                        )
                    mv = stat.tile([P, 2], F32)
                    nc.vector.bn_aggr(out=mv[:], in_=stt[:])
                    std = stat.tile([P, 1], F32)
                    nc.scalar.activation(
                        std[:], mv[:, 1:2], AF.Sqrt, bias=eps_sb[:], scale=1.0
                    )
                    rstd = stat.tile([P, 1], F32)
                    nc.vector.reciprocal(rstd[:], std[:])
                    nmr = stat.tile([P, 1], F32)
                    nc.vector.scalar_tensor_tensor(
                        out=nmr[:], in0=mv[:, 0:1], scalar=-1.0, in1=rstd[:],
                        op0=OP.mult, op1=OP.mult,
                    )
                    t0 = tpool.tile([P, D], F16)
                    nc.scalar.activation(
                        t0[:], z_ps[:], AF.Identity, bias=nmr[:], scale=rstd[:]
                    )
                    nc.vector.tensor_mul(out=t0[:], in0=t0[:], in1=gamma_sb[:])
                    nc.vector.tensor_add(out=t0[:], in0=t0[:], in1=beta_sb[:])
                    u_sb = wpool.tile([P, D], F16)
                    nc.vector.tensor_add(out=u_sb[:], in0=t0[:], in1=y_t)
                    nc.vector.tensor_mul(out=o_sb[:, t, :], in0=u_sb[:], in1=y_t)

                if st == nst - 1:
                    # last chunk: per-tile stores to shorten the drain tail
                    nc.scalar.dma_start(
                        out=outh[st, :, bass.ts(t, D)], in_=o_sb[:, t, :]
                    )
            if st < nst - 1:
                nc.scalar.dma_start(
                    out=outh[st].rearrange("p (t o) -> p t o", t=TPC), in_=o_sb[:]
                )

    nc.finalize()
    _BUILD_CACHE[key] = nc
    return nc


def _run(nc, in_maps, **kwargs):
    return bass_utils.run_bass_kernel_spmd(
        nc, in_maps, core_ids=list(range(N_CORES)), **kwargs
    )


def _prepare(x, y, weight, bias, gamma, beta):
    x = np.asarray(x, dtype=np.float32)
    y = np.ascontiguousarray(y, dtype=np.float32)
    weight = np.asarray(weight, dtype=np.float32)
    bias = np.asarray(bias, dtype=np.float32)
    gamma = np.asarray(gamma, dtype=np.float32)
    beta = np.asarray(beta, dtype=np.float32)

    B, IN = x.shape
    assert IN == D and weight.shape == (D, D) and y.shape == (B, D)
    assert B % (N_CORES * ST) == 0
    b_core = B // N_CORES
    nst = b_core // ST

    trivial = bool(np.all(gamma == 1.0)) and bool(np.all(beta == 0.0))
    nc = _build(b_core, trivial)

    fp8 = ml_dtypes.float8_e4m3fn
    # W.T packed fp8: wth[p, k*D + o] = 32 * W.T[k*P+p, o] = 32 * weight[o, k*P+p]
    wt = (weight.T * W_SCALE).astype(fp8)            # [i, o] = [k*P+p, o]
    wth_prep = np.ascontiguousarray(
        wt.reshape(KT, P, D).transpose(1, 0, 2)
    ).reshape(P, KT * D)
    biash = (bias * W_SCALE).astype(np.float16)
    in_maps = []
    for cid in range(N_CORES):
        xs = x[cid * b_core : (cid + 1) * b_core].astype(fp8)
        # x.T packed fp8: xt[st, p, k*ST + b] = x.T[k*P+p, st*ST + b]
        xt_prep = np.ascontiguousarray(
            xs.T.reshape(KT, P, nst, ST).transpose(2, 1, 0, 3)
        ).reshape(nst, P, KT * ST)
        ys = y[cid * b_core : (cid + 1) * b_core].astype(np.float16)
        # y packed: yh[st, p, t*D + o] = y[st*ST + t*P + p, o]
        yh_prep = np.ascontiguousarray(
            ys.reshape(nst, TPC, P, D).transpose(0, 2, 1, 3)
        ).reshape(nst, P, TPC * D)
        m = {
            "xt": xt_prep,
            "yh": yh_prep,
            "wth": wth_prep,
            "biash": biash,
        }
        if not trivial:
            m["gamma"] = gamma
            m["beta"] = beta
        in_maps.append(m)
    return nc, in_maps


def kernel(x, y, weight, bias, gamma, beta):
    nc, in_maps = _prepare(x, y, weight, bias, gamma, beta)
    res = _run(nc, in_maps)
    B = np.asarray(x).shape[0]
    b_core = B // N_CORES
    nst = b_core // ST
    outs = []
    for r in res.results:
        # unpack outh[st, p, t*D + o] -> out[st*ST + t*P + p, o]
        oh = np.asarray(r["outh"]).astype(np.float32)
        oh = oh.reshape(nst, P, TPC, D).transpose(0, 2, 1, 3).reshape(b_core, D)
        outs.append(oh)
    return np.concatenate(outs, axis=0)
